# revision 51
# baseline (speedup 1.0000x reference)
"""Fused multi-head attention (LN + QKV + softmax + out-proj) for TRN2,
sharded over 8 NeuronCores: batch (4) x head-group (2 groups of 6 heads).

Per core, for its (batch, head-group) shard (matmuls bf16, f32 PSUM):
  phase 1: x loaded in 2-tile batches on the Sync HWDGE ring; LayerNorm
    normalize on ScalarE (Identity activation, per-partition scale=rstd
    bias=-mu*rstd; stats on Vector), xn^T via xbar DMA transpose (no PE
    transposes, no PSUM eviction copies); V = xnT.T @ Wv per tile;
    Q^T,K^T = W @ xnT per 512-token chunk (pair-packed: head 2i in
    partitions 0:64, 2i+1 in 64:128), plus partition-swapped duplicates
    (SBUF DMA on the GpSimd SWDGE queue) so odd key-tiles contract the
    other PE row half - their stationary loads overlap the running
    matmul (measured +38us without this).
  phase 2: per (head, 1024-query block) unit, software-pipelined kt
    loop: S^T matmuls into a 2-deep PSUM pool, exp on ScalarE (12 kts)
    or the custom DVE exp2 pair (4 kts, score tiles parked in rotating
    psV-tag slots), each PV matmul lagged ONE act step behind its exp so
    the PE never waits on the exp chain; the next unit's first score
    matmul is pre-emitted before this unit's last PV (handoff) so the
    PE rides through unit boundaries. Denominators ride as PSUM row 64
    ([V|1] stationary); normalize = row-64 copy + fast reciprocal (DVE),
    gpsimd partition-broadcast, then a multiply (DVE, reads PV straight
    from PSUM) deferred into the NEXT unit so the broadcast latency
    hides. Odd heads bounce their normalized half via DMA to the upper
    partitions of att^T. The final unit runs fewer DVE key-tiles so the
    DVE queue drains before the last normalize.
  phase 3: out = att^T.T @ WoT per 128-token tile (PSUM 4-deep),
    evictions split Vector/Scalar, DMA out.
Host sums the two head-group partials per batch.

NOTE on measurement: the shared trn2 device is bimodal - sustained load
throttles it ~18% (424-434us vs 357-362us for this kernel). Compare
configs only on a cooled device (>=2-3min idle) or interleaved A/B.
"""
import numpy as np

import concourse.bass as bass
import concourse.bacc as bacc
import concourse.tile as tile
from concourse import mybir
from concourse.bass_utils import run_bass_kernel_spmd

F32 = mybir.dt.float32
BF16 = mybir.dt.bfloat16
AF = mybir.ActivationFunctionType
ALU = mybir.AluOpType

# ---- custom DVE exp2 (offloads part of the softmax exp from ScalarE) ----
# Scores arrive pre-scaled by log2(e) (folded into Wq on the host), so
# exp(s) = 2^y. Two DVE instructions at 1 elem/cycle each:
#   EXP2_BITS: k = round(y) via the +1.5*2^23 trick; writes (k+127)*2^23
#              to an int32 tile -> its bit pattern is the float 2^k.
#   EXP2_FRAC: f = y - round(y) in [-0.5, 0.5]; out = 2^k * (1 + f*(a + f*b))
# max rel err ~2e-3 (minimax quadratic for 2^f with the constant term fixed).
_RBIAS = 12582912.0          # 1.5 * 2^23
_PA, _PB = 0.70294179, 0.23986403
LN2 = 0.6931471805599453
LOG2E = 1.4426950408889634

_EXP_OPS = {}


def _register_exp_ops():
    if _EXP_OPS:
        return _EXP_OPS
    from concourse import dve_ops
    from concourse.dve_spec import Spec, Src0, Src1, C0, C1, C2, One, lower
    from concourse.dve_spec import _has_src1
    from concourse.dve_uop import DveOpSpec

    def _ref_bits(in0, in1, c0, c1, c2):
        y = in0.astype(np.float32)
        t = (y + np.float32(c0)).astype(np.float32)
        k = (t - np.float32(c0)).astype(np.float32)
        return (k * np.float32(c1) + np.float32(c2)).astype(np.float32)

    def _ref_frac(in0, in1, c0, c1, c2):
        y = in0.astype(np.float32)
        t = (y + np.float32(c0)).astype(np.float32)
        k = (t - np.float32(c0)).astype(np.float32)
        f = (y - k).astype(np.float32)
        return in1 * (1 + f * (np.float32(c1) + f * np.float32(c2)))

    t = Src0 + C0
    bits_body = (t - C0) * C1 + C2
    t2 = Src0 + C0
    f = Src0 - (t2 - C0)
    frac_body = Src1 * (One + f * (C1 + f * C2))

    ops = []
    for name, body, ref in (("EXP2_BITS_ATT", bits_body, _ref_bits),
                            ("EXP2_FRAC_ATT", frac_body, _ref_frac)):
        op = dve_ops.DveOp(name, Spec(body=body, reference=ref),
                           subdim=False, uops_sha={})
        dve_ops.OPS.append(op)
        dve_ops.CUSTOM_DVE_SPECS[name] = op.spec
        opcode = dve_ops._CUSTOM_DVE_ROW_BASE + len(dve_ops.OPS) - 1
        dve_ops._SUB_OPCODE_FOR_NAME[name] = opcode
        for ver in ("v3", "v4"):
            uops = lower(op.spec, ver=ver)
            op.uops_sha[ver] = DveOpSpec(
                name=name, opcode=opcode, uops=uops,
                rd1_en=_has_src1(op.spec)).sha(ver)
        ops.append(op)
    _EXP_OPS["bits"], _EXP_OPS["frac"] = ops
    return _EXP_OPS


B, N, DIM, H, DH = 4, 2048, 768, 12, 64
NCORES = 8
NH = 6            # heads per core
NP = 3            # head pairs per core
HCOLS = NH * DH   # 384

QHW = 1024        # query-block width (wide engine ops amortize fixed costs)
# key-tiles whose exp runs on the DVE (custom exp2 pair) instead of ScalarE;
# their score tiles borrow idle psV slots (the pv accumulator pins one slot,
# the other rotates through the DVE scratch tiles)
DVE_KTS = (2, 5, 8, 11)
# act-loop index -> DVE chain emission / deferred-PV emission
DVE_EMIT_J = {0: 0, 3: 1, 6: 2, 9: 3}
DVE_PV_J = {3: 0, 6: 1, 9: 2, 11: 3}
USE_SWAP = True    # odd key-tiles read partition-swapped K/Q copies: their
                   # stationary loads go to the other PE row half and overlap
                   # the running matmul (measured +38us when disabled)


def build_graph(n=N, dim=DIM, num_devices=NCORES):
    nt = n // 128        # token/key tiles
    ncdm = dim // 128    # dmodel chunks
    nqh = n // QHW       # query blocks

    nc = bacc.Bacc("TRN2", target_bir_lowering=False, debug=False,
                   num_devices=num_devices)
    x = nc.dram_tensor("x", [n, dim], F32, kind="ExternalInput").ap()
    wqt = nc.dram_tensor("wqt", [dim, HCOLS], BF16, kind="ExternalInput").ap()
    wkt = nc.dram_tensor("wkt", [dim, HCOLS], BF16, kind="ExternalInput").ap()
    wvt = nc.dram_tensor("wvt", [dim, HCOLS], BF16, kind="ExternalInput").ap()
    wot = nc.dram_tensor("wot", [HCOLS, dim], BF16, kind="ExternalInput").ap()
    out = nc.dram_tensor("out", [n, dim], BF16, kind="ExternalOutput").ap()

    import os
    dbg = {}
    if os.environ.get("KDBG", "0") == "1":
        dbg["xnT"] = nc.dram_tensor("d_xnT", [128, ncdm, n], BF16,
                                    kind="ExternalOutput").ap()
        dbg["kt"] = nc.dram_tensor("d_kt", [128, NP, n], BF16,
                                   kind="ExternalOutput").ap()
        dbg["qt"] = nc.dram_tensor("d_qt", [128, NP, n], BF16,
                                   kind="ExternalOutput").ap()
        dbg["v"] = nc.dram_tensor("d_v", [128, NH, nt, DH + 1], BF16,
                                  kind="ExternalOutput").ap()
        dbg["att"] = nc.dram_tensor("d_att", [128, NP, n], BF16,
                                    kind="ExternalOutput").ap()

    with tile.TileContext(nc) as tc:
        _body(tc, x, wqt, wkt, wvt, wot, out, n, dim, nt, ncdm, nqh, dbg)
    nc.compile()
    return nc


def _body(tc, x, wqt, wkt, wvt, wot, out, n, dim, nt, ncdm, nqh, dbg=None):
    nc = tc.nc
    qhw = QHW
    from contextlib import ExitStack
    with ExitStack() as ctx:
        consts = ctx.enter_context(tc.tile_pool(name="consts", bufs=1))
        sb = ctx.enter_context(tc.tile_pool(name="sb", bufs=1))
        xfp = ctx.enter_context(tc.tile_pool(name="xfp", bufs=6))
        xpool = ctx.enter_context(tc.tile_pool(name="xp", bufs=4))
        small = ctx.enter_context(tc.tile_pool(name="small", bufs=4))
        ppool = ctx.enter_context(tc.tile_pool(name="pp", bufs=4))
        rbpool = ctx.enter_context(tc.tile_pool(name="rb", bufs=1))
        oddp = ctx.enter_context(tc.tile_pool(name="odd", bufs=2))
        otp = ctx.enter_context(tc.tile_pool(name="ot", bufs=3))
        bitp = ctx.enter_context(tc.tile_pool(name="bitp", bufs=2))

        # x loaded in 2-tile batches (halves the DMA dispatch count) on the
        # Sync HWDGE ring; weights + swap copies go via the GpSimd SWDGE
        # queue. (SWDGE advances at transfer rate, so bulk x there starves
        # the queue; transposes + x together on Sync head-of-line block.)
        x4 = x.rearrange("(t two p) d -> t p two d", two=2, p=128)
        out3 = out.rearrange("(t p) d -> t p d", p=128)

        eps_sb = consts.tile([128, 1], F32, tag="eps")
        nc.vector.memset(eps_sb, 1e-5)
        xpairs = []
        for tp in range(nt // 2):
            xp_ = xfp.tile([128, 2, dim], F32, tag="xf")
            xpairs.append(xp_)
        xtiles = [xpairs[tt // 2][:, tt % 2, :] for tt in range(nt)]
        # Sync HWDGE ring order: 2 x pairs, the weights, remaining x pairs
        for tp in range(2):
            nc.sync.dma_start(out=xpairs[tp], in_=x4[tp])
        wv_sb = consts.tile([128, ncdm, HCOLS], BF16, tag="wv")
        nc.sync.dma_start(out=wv_sb, in_=wvt.rearrange("(c p) m -> p c m", p=128))
        wk_sb = consts.tile([128, ncdm, HCOLS], BF16, tag="wk")
        nc.scalar.dma_start(out=wk_sb, in_=wkt.rearrange("(c p) m -> p c m", p=128))
        wq_sb = consts.tile([128, ncdm, HCOLS], BF16, tag="wq")
        nc.scalar.dma_start(out=wq_sb, in_=wqt.rearrange("(c p) m -> p c m", p=128))
        wo_sb = consts.tile([128, NP, dim], BF16, tag="wo")
        nc.scalar.dma_start(out=wo_sb, in_=wot.rearrange("(c p) m -> p c m", p=128))
        for tp in range(2, nt // 2):
            nc.sync.dma_start(out=xpairs[tp], in_=x4[tp])

        # persistent activations. K^T/Q^T are pair-packed: pair i holds head
        # 2i in partitions 0:64 and head 2i+1 in 64:128 ("natural"); the *w
        # copies are partition-swapped duplicates (via SBUF->SBUF DMA) so a
        # head's stationary/moving operands exist in BOTH halves - odd key
        # tiles read the swapped copy, so their stationary loads go to the
        # other PE row half and overlap the running matmul.
        xnT = sb.tile([128, ncdm, n], BF16, tag="xnT")
        qt_sb = sb.tile([128, NP, n], BF16, tag="qt")
        kt_sb = sb.tile([128, NP, n], BF16, tag="kt")
        if USE_SWAP:
            qtw_sb = sb.tile([128, NP, n], BF16, tag="qtw")
            ktw_sb = sb.tile([128, NP, n], BF16, tag="ktw")
        v_sb = sb.tile([128, NH, nt, DH + 1], BF16, tag="v")
        nc.vector.memset(v_sb[:, :, :, DH:DH + 1], 1.0)
        att_sb = sb.tile([128, NP, n], BF16, tag="att")

        # ---- phase 1: LayerNorm + DMA transpose + Q/K/V projections ----
        with tc.tile_pool(name="psA", bufs=8, space="PSUM") as psA:
            for tt in range(nt):
                xt = xtiles[tt]
                stats = small.tile([128, 2, 6], F32, tag="stats")
                for g in range(2):
                    nc.vector.bn_stats(out=stats[:, g, :],
                                       in_=xt[:, g * 384:(g + 1) * 384])
                mv = small.tile([128, 2], F32, tag="mv")
                nc.vector.bn_aggr(out=mv, in_=stats)
                sq = small.tile([128, 1], F32, tag="sq")
                nc.scalar.activation(out=sq, in_=mv[:, 1:2], func=AF.Sqrt,
                                     bias=eps_sb)
                # -mu/sq and (in place) 1/sq in ONE gpsimd op: keeps the
                # small chain off Vector, whose in-order queue head-of-line
                # blocks on the next tile's DMA-gated bn_stats
                negmu = small.tile([128, 1], F32, tag="negmu")
                nc.scalar.activation(out=negmu, in_=mv[:, 0:1], func=AF.Copy,
                                     scale=-1.0)
                negb = small.tile([128, 1], F32, tag="negb")
                nc.gpsimd.normalize_recip(out_ap=negb, in_ap=negmu, denom_ap=sq)
                xn = xpool.tile([128, dim], BF16, tag="xn")
                nc.scalar.activation(out=xn, in_=xt, func=AF.Identity,
                                     scale=sq, bias=negb)
                # first tiles' transposes dispatch via the Scalar HWDGE
                # queue: on Sync they would sit behind all 8 x-pair loads
                # (ring backpressure), delaying the first PE matmul ~20us
                teng = nc.scalar if tt < 4 else nc.sync
                teng.dma_start_transpose(
                    out=xnT[:, :, tt * 128:(tt + 1) * 128], in_=xn)
                pst = psA.tile([128, 512], F32, tag="psA")
                for c in range(ncdm):
                    nc.tensor.matmul(pst[:, 0:HCOLS],
                                     xnT[:, c, tt * 128:(tt + 1) * 128],
                                     wv_sb[:, c, :],
                                     start=(c == 0), stop=(c == ncdm - 1))
                nc.scalar.copy(
                    out=v_sb[:, :, tt, 0:DH],
                    in_=pst[:, 0:HCOLS].rearrange("p (s d) -> p s d", d=DH))
                # K/Q projections for each completed 512-token column chunk;
                # K first (with its swap DMAs) so phase 2 can start sooner.
                if tt % 4 == 3:
                    cc = tt // 4
                    csl = slice(cc * 512, (cc + 1) * 512)
                    for i in range(NP):
                        pst = psA.tile([128, 512], F32, tag="psA")
                        for c in range(ncdm):
                            nc.tensor.matmul(pst,
                                             wk_sb[:, c, i * 128:(i + 1) * 128],
                                             xnT[:, c, csl],
                                             start=(c == 0), stop=(c == ncdm - 1))
                        if i % 2 == 0:
                            nc.scalar.copy(out=kt_sb[:, i, csl], in_=pst)
                        else:
                            nc.vector.tensor_copy(out=kt_sb[:, i, csl], in_=pst)
                    if USE_SWAP:
                        nc.gpsimd.dma_start(out=ktw_sb[64:128, :, csl],
                                            in_=kt_sb[0:64, :, csl])
                        nc.gpsimd.dma_start(out=ktw_sb[0:64, :, csl],
                                            in_=kt_sb[64:128, :, csl])
                    for i in range(NP):
                        pst = psA.tile([128, 512], F32, tag="psA")
                        for c in range(ncdm):
                            nc.tensor.matmul(pst,
                                             wq_sb[:, c, i * 128:(i + 1) * 128],
                                             xnT[:, c, csl],
                                             start=(c == 0), stop=(c == ncdm - 1))
                        if i % 2 == 0:
                            nc.vector.tensor_copy(out=qt_sb[:, i, csl], in_=pst)
                        else:
                            nc.scalar.copy(out=qt_sb[:, i, csl], in_=pst)
                    if USE_SWAP:
                        nc.gpsimd.dma_start(out=qtw_sb[64:128, :, csl],
                                            in_=qt_sb[0:64, :, csl])
                        nc.gpsimd.dma_start(out=qtw_sb[0:64, :, csl],
                                            in_=qt_sb[64:128, :, csl])

        # ---- phase 2: attention ----
        eo = _register_exp_ops()
        from contextlib import ExitStack as _ES
        with _ES() as p2:
            psV = p2.enter_context(
                tc.tile_pool(name="psV", bufs=2, space="PSUM"))
            psS_cm = tc.tile_pool(name="psS", bufs=2, space="PSUM")
            psS = psS_cm.__enter__()

            pending = [None]

            def flush_mul():
                pv_, rc_, i_, s_, q0_ = pending[0]
                if s_ == 0:
                    nc.vector.tensor_mul(out=att_sb[0:64, i_, q0_:q0_ + qhw],
                                         in0=pv_[0:64, :], in1=rc_)
                else:
                    tmp = oddp.tile([64, qhw], BF16, tag="odd")
                    nc.vector.tensor_mul(out=tmp, in0=pv_[0:64, :], in1=rc_)
                    nc.sync.dma_start(out=att_sb[64:128, i_, q0_:q0_ + qhw],
                                      in_=tmp)
                pending[0] = None

            def scores_mm_for(h, q0, sc, kt):
                i, s = h // 2, h % 2
                if kt % 2 == 0 or not USE_SWAP:
                    lh, rh = kt_sb, qt_sb
                    half = slice(64 * s, 64 * s + 64)
                else:
                    lh, rh = ktw_sb, qtw_sb
                    half = slice(64 * (1 - s), 64 * (1 - s) + 64)
                for qq in range(qhw // 512):
                    nc.tensor.matmul(
                        sc[:, qq * 512:(qq + 1) * 512],
                        lh[half, i, kt * 128:(kt + 1) * 128],
                        rh[half, i, q0 + qq * 512:q0 + (qq + 1) * 512])

            def wo_job(t, pool, evict_all_vector=False):
                po = pool.tile([128, dim], F32, tag="pv" if pool is psV else "po")
                for c in range(NP):
                    lhsT = att_sb[:, c, t * 128:(t + 1) * 128]
                    # 512-col chunks: PSUM matmul outputs must not straddle
                    # a 2KB bank boundary
                    for o0 in (0, 512):
                        o1 = min(o0 + 512, dim)
                        nc.tensor.matmul(po[:, o0:o1], lhsT,
                                         wo_sb[:, c, o0:o1],
                                         start=(c == 0), stop=(c == NP - 1))
                ot = otp.tile([128, dim], out.dtype, tag="ot")
                # whole-tile eviction on ONE engine, alternating per tile:
                # the psO slot then frees after a single op (+1 sem), not
                # the max of two engines' halves
                if t % 2 == 0:
                    nc.vector.tensor_copy(out=ot, in_=po)
                else:
                    nc.scalar.copy(out=ot, in_=po)
                nc.sync.dma_start(out=out3[t], in_=ot)

            def unit(h, qh, dve_kts, dve_emit_j, dve_pv_j, handoff, nxt,
                     wo_tiles=()):
                i, s = h // 2, h % 2
                q0 = qh * qhw
                pv = psV.tile([65, qhw], F32, tag="pv")

                def pv_mm(p_t, kt, start, stop):
                    for qq in range(qhw // 512):
                        nc.tensor.matmul(
                            pv[:, qq * 512:(qq + 1) * 512],
                            v_sb[:, h, kt, :],
                            p_t[:, qq * 512:(qq + 1) * 512],
                            start=start, stop=stop)

                dve_pts = {}

                def emit_dve(kt):
                    sc = psV.tile([128, qhw], F32, tag="pv")
                    scores_mm_for(h, q0, sc, kt)
                    bt = bitp.tile([128, qhw], mybir.dt.int32, tag="bits")
                    nc.vector._custom_dve(eo["bits"], out=bt, in0=sc,
                                          s0=_RBIAS, s1=8388608.0,
                                          imm2=1065353216.0)
                    p_t = ppool.tile([128, qhw], BF16, tag="pd", bufs=2)
                    nc.vector._custom_dve(eo["frac"], out=p_t, in0=sc,
                                          in1=bt[:].bitcast(F32),
                                          s0=_RBIAS, s1=_PA, imm2=_PB)
                    dve_pts[kt] = p_t

                acts = [k for k in range(nt) if k not in dve_kts]
                prev = None
                for j, kt in enumerate(acts):
                    if j in dve_emit_j:
                        emit_dve(dve_kts[dve_emit_j[j]])
                    if j == 0 and handoff is not None:
                        sc = handoff
                    else:
                        sc = psS.tile([128, qhw], F32, tag="sc")
                        scores_mm_for(h, q0, sc, kt)
                    if j == 0 and pending[0] is not None:
                        flush_mul()
                    if j in dve_pv_j:
                        dkt = dve_kts[dve_pv_j[j]]
                        pv_mm(dve_pts.pop(dkt), dkt, start=False, stop=False)
                    if prev is not None:
                        pp, pkt = prev
                        pv_mm(pp, pkt, start=(j == 1), stop=False)
                    if wo_tiles and j in (4, 8):
                        widx = 0 if j == 4 else 1
                        if widx < len(wo_tiles):
                            wo_job(wo_tiles[widx], psV, evict_all_vector=True)
                    p_t = ppool.tile([128, qhw], BF16, tag="p")
                    nc.scalar.activation(out=p_t, in_=sc, func=AF.Exp,
                                         scale=LN2)
                    prev = (p_t, kt)
                assert not dve_pts
                # pre-emit the NEXT unit's first score matmul so the PE has
                # work while this unit's last exp drains
                handoff_out = None
                if nxt is not None:
                    nh_, nqh_, nkt0 = nxt
                    handoff_out = psS.tile([128, qhw], F32, tag="sc")
                    scores_mm_for(nh_, nqh_ * qhw, handoff_out, nkt0)
                pp, pkt = prev
                pv_mm(pp, pkt, start=False, stop=True)
                # denominator row 64 -> partition 0 via plain copy (custom
                # DVE ops can't partition-shift PSUM reads), reciprocal; the
                # multiply is deferred into the next unit so the gpsimd
                # broadcast latency hides behind that unit's first exps
                srow = rbpool.tile([1, qhw], F32, tag="srow")
                nc.vector.tensor_copy(out=srow, in_=pv[64:65, :])
                rrow = rbpool.tile([1, qhw], F32, tag="rrow")
                nc.vector.reciprocal_approx_fast(out=rrow, in_=srow)
                rc = rbpool.tile([64, qhw], F32, tag="rc", bufs=2)
                nc.gpsimd.partition_broadcast(rc, rrow)
                pending[0] = (pv, rc, i, s, q0)
                return handoff_out

            units = [(h, qh) for h in range(NH) for qh in range(nqh)]
            wo_sched = {}  # in-unit Wo jobs measured slower (psV contention)
            # the final unit runs fewer DVE key-tiles so the DVE queue is
            # drained by the time phase 3 needs the last normalize multiply
            cfg_main = (DVE_KTS, DVE_EMIT_J, DVE_PV_J)
            cfg_last = ((2, 5), {0: 0, 3: 1}, {3: 0, 6: 1})
            handoff = None
            for u, (h, qh) in enumerate(units):
                dkts, dej, dpj = cfg_last if u == len(units) - 1 else cfg_main
                nxt = None
                if u + 1 < len(units):
                    nh_, nqh_ = units[u + 1]
                    ndkts = cfg_last[0] if u + 1 == len(units) - 1 else DVE_KTS
                    nacts = [k for k in range(nt) if k not in ndkts]
                    nxt = (nh_, nqh_, nacts[0])
                handoff = unit(h, qh, dkts, dej, dpj, handoff, nxt,
                               wo_tiles=wo_sched.get(u, ()))
            # close the score pool BEFORE the final deferred multiply: its
            # close-barrier only waits on the last exp, so its 4 banks are
            # free for the output projection while psV (the pv accumulator)
            # is still pinned by the final normalize - Wo tiles 0..14 don't
            # need the last head's data and start immediately
            psS_cm.__exit__(None, None, None)
            flush_mul()

            # ---- phase 3: output projection (2-deep in the freed banks) ----
            with tc.tile_pool(name="psO", bufs=2, space="PSUM") as psO:
                for t in range(nt):
                    wo_job(t, psO)
                if dbg:
                    nc.sync.dma_start(out=dbg["xnT"], in_=xnT)
                    nc.sync.dma_start(out=dbg["kt"], in_=kt_sb)
                    nc.sync.dma_start(out=dbg["qt"], in_=qt_sb)
                    nc.sync.dma_start(out=dbg["v"], in_=v_sb)
                    nc.sync.dma_start(out=dbg["att"], in_=att_sb)


_NC_CACHE = {}


def _get_nc():
    if "nc" not in _NC_CACHE:
        _NC_CACHE["nc"] = build_graph()
    return _NC_CACHE["nc"]


def make_in_maps(x, gamma, Wq, Wk, Wv, Wo):
    """Host-side sharding: core c -> batch c//2, head-group c%2."""
    import ml_dtypes
    bf16 = ml_dtypes.bfloat16
    g = (np.asarray(gamma, np.float32) + 1.0)
    scale = DH ** -0.5 * LOG2E  # scores computed in log2 domain
    Wq_eff = np.asarray(Wq, np.float32) * g[None, :] * scale
    Wk_eff = np.asarray(Wk, np.float32) * g[None, :]
    Wv_eff = np.asarray(Wv, np.float32)
    Wo_eff = np.asarray(Wo, np.float32)
    hg_maps = []
    for hg in range(2):
        r0, r1 = hg * HCOLS, (hg + 1) * HCOLS
        hg_maps.append({
            "wqt": np.ascontiguousarray(Wq_eff[r0:r1, :].T).astype(bf16),
            "wkt": np.ascontiguousarray(Wk_eff[r0:r1, :].T).astype(bf16),
            "wvt": np.ascontiguousarray(Wv_eff[r0:r1, :].T).astype(bf16),
            "wot": np.ascontiguousarray(Wo_eff[:, r0:r1].T).astype(bf16),
        })
    in_maps = []
    for c in range(NCORES):
        b, hg = c // 2, c % 2
        m = dict(hg_maps[hg])
        m["x"] = np.ascontiguousarray(np.asarray(x, np.float32)[b])
        in_maps.append(m)
    return in_maps


def _run(inputs, trace=False, trace_kwargs=None):
    nc = _get_nc()
    in_maps = make_in_maps(**inputs)
    res = run_bass_kernel_spmd(nc, in_maps, core_ids=list(range(NCORES)),
                               trace=trace, **(trace_kwargs or {}))
    out = np.empty((B, N, DIM), np.float32)
    for b in range(B):
        out[b] = (res.results[2 * b]["out"].astype(np.float32)
                  + res.results[2 * b + 1]["out"].astype(np.float32))
    return out, res


def kernel(x, gamma, Wq, Wk, Wv, Wo):
    out, _ = _run(dict(x=x, gamma=gamma, Wq=Wq, Wk=Wk, Wv=Wv, Wo=Wo))
    return out


# revision 52
# speedup vs baseline: 1.0288x; 1.0288x over previous
"""Fused multi-head attention (LN + QKV + softmax + out-proj) for TRN2,
sharded over 8 NeuronCores: batch (4) x head-group (2 groups of 6 heads).

Per core, for its (batch, head-group) shard (matmuls bf16, f32 PSUM):
  phase 1: x loaded in 2-tile batches on the Sync HWDGE ring; LayerNorm
    normalize on ScalarE (Identity activation, per-partition scale=rstd
    bias=-mu*rstd; stats on Vector), xn^T via xbar DMA transpose (no PE
    transposes, no PSUM eviction copies); V = xnT.T @ Wv per tile;
    Q^T,K^T = W @ xnT per 512-token chunk (pair-packed: head 2i in
    partitions 0:64, 2i+1 in 64:128), plus partition-swapped duplicates
    (SBUF DMA on the GpSimd SWDGE queue) so odd key-tiles contract the
    other PE row half - their stationary loads overlap the running
    matmul (measured +38us without this).
  phase 2: per (head, 1024-query block) unit, software-pipelined kt
    loop: S^T matmuls into a 2-deep PSUM pool, exp on ScalarE (12 kts)
    or the custom DVE exp2 pair (4 kts, score tiles parked in rotating
    psV-tag slots), each PV matmul lagged ONE act step behind its exp so
    the PE never waits on the exp chain; the next unit's first score
    matmul is pre-emitted before this unit's last PV (handoff) so the
    PE rides through unit boundaries. Denominators ride as PSUM row 64
    ([V|1] stationary); normalize = row-64 copy + fast reciprocal (DVE),
    gpsimd partition-broadcast, then a multiply (DVE, reads PV straight
    from PSUM) deferred into the NEXT unit so the broadcast latency
    hides. Odd heads bounce their normalized half via DMA to the upper
    partitions of att^T. The final unit runs fewer DVE key-tiles so the
    DVE queue drains before the last normalize.
  phase 3: out = att^T.T @ WoT per 128-token tile (PSUM 4-deep),
    evictions split Vector/Scalar, DMA out.
Host sums the two head-group partials per batch.

NOTE on measurement: the shared trn2 device is bimodal - sustained load
throttles it ~18% (424-434us vs 357-362us for this kernel). Compare
configs only on a cooled device (>=2-3min idle) or interleaved A/B.
"""
import numpy as np

import concourse.bass as bass
import concourse.bacc as bacc
import concourse.tile as tile
from concourse import mybir
from concourse.bass_utils import run_bass_kernel_spmd

F32 = mybir.dt.float32
BF16 = mybir.dt.bfloat16
AF = mybir.ActivationFunctionType
ALU = mybir.AluOpType

# ---- custom DVE exp2 (offloads part of the softmax exp from ScalarE) ----
# Scores arrive pre-scaled by log2(e) (folded into Wq on the host), so
# exp(s) = 2^y. Two DVE instructions at 1 elem/cycle each:
#   EXP2_BITS: k = round(y) via the +1.5*2^23 trick; writes (k+127)*2^23
#              to an int32 tile -> its bit pattern is the float 2^k.
#   EXP2_FRAC: f = y - round(y) in [-0.5, 0.5]; out = 2^k * (1 + f*(a + f*b))
# max rel err ~2e-3 (minimax quadratic for 2^f with the constant term fixed).
_RBIAS = 12582912.0          # 1.5 * 2^23
_PA, _PB = 0.70294179, 0.23986403
LN2 = 0.6931471805599453
LOG2E = 1.4426950408889634

_EXP_OPS = {}


def _register_exp_ops():
    if _EXP_OPS:
        return _EXP_OPS
    from concourse import dve_ops
    from concourse.dve_spec import Spec, Src0, Src1, C0, C1, C2, One, lower
    from concourse.dve_spec import _has_src1
    from concourse.dve_uop import DveOpSpec

    def _ref_bits(in0, in1, c0, c1, c2):
        y = in0.astype(np.float32)
        t = (y + np.float32(c0)).astype(np.float32)
        k = (t - np.float32(c0)).astype(np.float32)
        return (k * np.float32(c1) + np.float32(c2)).astype(np.float32)

    def _ref_frac(in0, in1, c0, c1, c2):
        y = in0.astype(np.float32)
        t = (y + np.float32(c0)).astype(np.float32)
        k = (t - np.float32(c0)).astype(np.float32)
        f = (y - k).astype(np.float32)
        return in1 * (1 + f * (np.float32(c1) + f * np.float32(c2)))

    t = Src0 + C0
    bits_body = (t - C0) * C1 + C2
    t2 = Src0 + C0
    f = Src0 - (t2 - C0)
    frac_body = Src1 * (One + f * (C1 + f * C2))

    ops = []
    for name, body, ref in (("EXP2_BITS_ATT", bits_body, _ref_bits),
                            ("EXP2_FRAC_ATT", frac_body, _ref_frac)):
        op = dve_ops.DveOp(name, Spec(body=body, reference=ref),
                           subdim=False, uops_sha={})
        dve_ops.OPS.append(op)
        dve_ops.CUSTOM_DVE_SPECS[name] = op.spec
        opcode = dve_ops._CUSTOM_DVE_ROW_BASE + len(dve_ops.OPS) - 1
        dve_ops._SUB_OPCODE_FOR_NAME[name] = opcode
        for ver in ("v3", "v4"):
            uops = lower(op.spec, ver=ver)
            op.uops_sha[ver] = DveOpSpec(
                name=name, opcode=opcode, uops=uops,
                rd1_en=_has_src1(op.spec)).sha(ver)
        ops.append(op)
    _EXP_OPS["bits"], _EXP_OPS["frac"] = ops
    return _EXP_OPS


B, N, DIM, H, DH = 4, 2048, 768, 12, 64
NCORES = 8
NH = 6            # heads per core
NP = 3            # head pairs per core
HCOLS = NH * DH   # 384

QHW = 1024        # query-block width (wide engine ops amortize fixed costs)
# key-tiles whose exp runs on the DVE (custom exp2 pair) instead of ScalarE;
# their score tiles borrow idle psV slots (the pv accumulator pins one slot,
# the other rotates through the DVE scratch tiles)
DVE_KTS = (2, 5, 8, 11)
# act-loop index -> DVE chain emission / deferred-PV emission
DVE_EMIT_J = {0: 0, 3: 1, 6: 2, 9: 3}
DVE_PV_J = {3: 0, 6: 1, 9: 2, 11: 3}
USE_SWAP = True    # odd key-tiles read partition-swapped K/Q copies: their
                   # stationary loads go to the other PE row half and overlap
                   # the running matmul (measured +38us when disabled)


def build_graph(n=N, dim=DIM, num_devices=NCORES):
    nt = n // 128        # token/key tiles
    ncdm = dim // 128    # dmodel chunks
    nqh = n // QHW       # query blocks

    nc = bacc.Bacc("TRN2", target_bir_lowering=False, debug=False,
                   num_devices=num_devices)
    x = nc.dram_tensor("x", [n, dim], F32, kind="ExternalInput").ap()
    wqt = nc.dram_tensor("wqt", [dim, HCOLS], BF16, kind="ExternalInput").ap()
    wkt = nc.dram_tensor("wkt", [dim, HCOLS], BF16, kind="ExternalInput").ap()
    wvt = nc.dram_tensor("wvt", [dim, HCOLS], BF16, kind="ExternalInput").ap()
    wot = nc.dram_tensor("wot", [HCOLS, dim], BF16, kind="ExternalInput").ap()
    out = nc.dram_tensor("out", [n, dim], BF16, kind="ExternalOutput").ap()

    import os
    dbg = {}
    if os.environ.get("KDBG", "0") == "1":
        dbg["xnT"] = nc.dram_tensor("d_xnT", [128, ncdm, n], BF16,
                                    kind="ExternalOutput").ap()
        dbg["kt"] = nc.dram_tensor("d_kt", [128, NP, n], BF16,
                                   kind="ExternalOutput").ap()
        dbg["qt"] = nc.dram_tensor("d_qt", [128, NP, n], BF16,
                                   kind="ExternalOutput").ap()
        dbg["v"] = nc.dram_tensor("d_v", [128, NH, nt, DH + 1], BF16,
                                  kind="ExternalOutput").ap()
        dbg["att"] = nc.dram_tensor("d_att", [128, NP, n], BF16,
                                    kind="ExternalOutput").ap()

    with tile.TileContext(nc) as tc:
        _body(tc, x, wqt, wkt, wvt, wot, out, n, dim, nt, ncdm, nqh, dbg)
    nc.compile()
    return nc


def _body(tc, x, wqt, wkt, wvt, wot, out, n, dim, nt, ncdm, nqh, dbg=None):
    nc = tc.nc
    qhw = QHW
    from contextlib import ExitStack
    with ExitStack() as ctx:
        consts = ctx.enter_context(tc.tile_pool(name="consts", bufs=1))
        sb = ctx.enter_context(tc.tile_pool(name="sb", bufs=1))
        xfp = ctx.enter_context(tc.tile_pool(name="xfp", bufs=6))
        xpool = ctx.enter_context(tc.tile_pool(name="xp", bufs=4))
        small = ctx.enter_context(tc.tile_pool(name="small", bufs=4))
        ppool = ctx.enter_context(tc.tile_pool(name="pp", bufs=4))
        rbpool = ctx.enter_context(tc.tile_pool(name="rb", bufs=1))
        oddp = ctx.enter_context(tc.tile_pool(name="odd", bufs=2))
        otp = ctx.enter_context(tc.tile_pool(name="ot", bufs=3))
        bitp = ctx.enter_context(tc.tile_pool(name="bitp", bufs=2))

        # x loaded in 2-tile batches (halves the DMA dispatch count) on the
        # Sync HWDGE ring; weights + swap copies go via the GpSimd SWDGE
        # queue. (SWDGE advances at transfer rate, so bulk x there starves
        # the queue; transposes + x together on Sync head-of-line block.)
        x4 = x.rearrange("(t two p) d -> t p two d", two=2, p=128)
        out3 = out.rearrange("(t p) d -> t p d", p=128)

        eps_sb = consts.tile([128, 1], F32, tag="eps")
        nc.vector.memset(eps_sb, 1e-5)
        xpairs = []
        for tp in range(nt // 2):
            xp_ = xfp.tile([128, 2, dim], F32, tag="xf")
            xpairs.append(xp_)
        xtiles = [xpairs[tt // 2][:, tt % 2, :] for tt in range(nt)]
        # Sync HWDGE ring order: 2 x pairs, the weights, remaining x pairs
        for tp in range(2):
            nc.sync.dma_start(out=xpairs[tp], in_=x4[tp])
        wv_sb = consts.tile([128, ncdm, HCOLS], BF16, tag="wv")
        nc.sync.dma_start(out=wv_sb, in_=wvt.rearrange("(c p) m -> p c m", p=128))
        wk_sb = consts.tile([128, ncdm, HCOLS], BF16, tag="wk")
        nc.sync.dma_start(out=wk_sb, in_=wkt.rearrange("(c p) m -> p c m", p=128))
        wq_sb = consts.tile([128, ncdm, HCOLS], BF16, tag="wq")
        nc.sync.dma_start(out=wq_sb, in_=wqt.rearrange("(c p) m -> p c m", p=128))
        wo_sb = consts.tile([128, NP, dim], BF16, tag="wo")
        nc.sync.dma_start(out=wo_sb, in_=wot.rearrange("(c p) m -> p c m", p=128))
        for tp in range(2, nt // 2):
            nc.sync.dma_start(out=xpairs[tp], in_=x4[tp])

        # persistent activations. K^T/Q^T are pair-packed: pair i holds head
        # 2i in partitions 0:64 and head 2i+1 in 64:128 ("natural"); the *w
        # copies are partition-swapped duplicates (via SBUF->SBUF DMA) so a
        # head's stationary/moving operands exist in BOTH halves - odd key
        # tiles read the swapped copy, so their stationary loads go to the
        # other PE row half and overlap the running matmul.
        xnT = sb.tile([128, ncdm, n], BF16, tag="xnT")
        qt_sb = sb.tile([128, NP, n], BF16, tag="qt")
        kt_sb = sb.tile([128, NP, n], BF16, tag="kt")
        if USE_SWAP:
            qtw_sb = sb.tile([128, NP, n], BF16, tag="qtw")
            ktw_sb = sb.tile([128, NP, n], BF16, tag="ktw")
        v_sb = sb.tile([128, NH, nt, DH + 1], BF16, tag="v")
        nc.vector.memset(v_sb[:, :, :, DH:DH + 1], 1.0)
        att_sb = sb.tile([128, NP, n], BF16, tag="att")

        # ---- phase 1: LayerNorm + DMA transpose + Q/K/V projections ----
        with tc.tile_pool(name="psA", bufs=8, space="PSUM") as psA:
            for tt in range(nt):
                xt = xtiles[tt]
                stats = small.tile([128, 2, 6], F32, tag="stats")
                for g in range(2):
                    nc.vector.bn_stats(out=stats[:, g, :],
                                       in_=xt[:, g * 384:(g + 1) * 384])
                mv = small.tile([128, 2], F32, tag="mv")
                nc.vector.bn_aggr(out=mv, in_=stats)
                sq = small.tile([128, 1], F32, tag="sq")
                nc.scalar.activation(out=sq, in_=mv[:, 1:2], func=AF.Sqrt,
                                     bias=eps_sb)
                # -mu/sq and (in place) 1/sq in ONE gpsimd op: keeps the
                # small chain off Vector, whose in-order queue head-of-line
                # blocks on the next tile's DMA-gated bn_stats
                negmu = small.tile([128, 1], F32, tag="negmu")
                nc.scalar.activation(out=negmu, in_=mv[:, 0:1], func=AF.Copy,
                                     scale=-1.0)
                negb = small.tile([128, 1], F32, tag="negb")
                nc.gpsimd.normalize_recip(out_ap=negb, in_ap=negmu, denom_ap=sq)
                xn = xpool.tile([128, dim], BF16, tag="xn")
                nc.scalar.activation(out=xn, in_=xt, func=AF.Identity,
                                     scale=sq, bias=negb)
                # first tiles' transposes dispatch via the Scalar HWDGE
                # queue: on Sync they would sit behind all 8 x-pair loads
                # (ring backpressure), delaying the first PE matmul ~20us
                teng = nc.scalar if tt < 4 else nc.sync
                teng.dma_start_transpose(
                    out=xnT[:, :, tt * 128:(tt + 1) * 128], in_=xn)
                pst = psA.tile([128, 512], F32, tag="psA")
                for c in range(ncdm):
                    nc.tensor.matmul(pst[:, 0:HCOLS],
                                     xnT[:, c, tt * 128:(tt + 1) * 128],
                                     wv_sb[:, c, :],
                                     start=(c == 0), stop=(c == ncdm - 1))
                nc.scalar.copy(
                    out=v_sb[:, :, tt, 0:DH],
                    in_=pst[:, 0:HCOLS].rearrange("p (s d) -> p s d", d=DH))
                # K/Q projections for each completed 512-token column chunk;
                # K first (with its swap DMAs) so phase 2 can start sooner.
                if tt % 4 == 3:
                    cc = tt // 4
                    csl = slice(cc * 512, (cc + 1) * 512)
                    for i in range(NP):
                        pst = psA.tile([128, 512], F32, tag="psA")
                        for c in range(ncdm):
                            nc.tensor.matmul(pst,
                                             wk_sb[:, c, i * 128:(i + 1) * 128],
                                             xnT[:, c, csl],
                                             start=(c == 0), stop=(c == ncdm - 1))
                        if i % 2 == 0:
                            nc.scalar.copy(out=kt_sb[:, i, csl], in_=pst)
                        else:
                            nc.vector.tensor_copy(out=kt_sb[:, i, csl], in_=pst)
                    if USE_SWAP:
                        nc.gpsimd.dma_start(out=ktw_sb[64:128, :, csl],
                                            in_=kt_sb[0:64, :, csl])
                        nc.gpsimd.dma_start(out=ktw_sb[0:64, :, csl],
                                            in_=kt_sb[64:128, :, csl])
                    for i in range(NP):
                        pst = psA.tile([128, 512], F32, tag="psA")
                        for c in range(ncdm):
                            nc.tensor.matmul(pst,
                                             wq_sb[:, c, i * 128:(i + 1) * 128],
                                             xnT[:, c, csl],
                                             start=(c == 0), stop=(c == ncdm - 1))
                        if i % 2 == 0:
                            nc.vector.tensor_copy(out=qt_sb[:, i, csl], in_=pst)
                        else:
                            nc.scalar.copy(out=qt_sb[:, i, csl], in_=pst)
                    if USE_SWAP:
                        nc.gpsimd.dma_start(out=qtw_sb[64:128, :, csl],
                                            in_=qt_sb[0:64, :, csl])
                        nc.gpsimd.dma_start(out=qtw_sb[0:64, :, csl],
                                            in_=qt_sb[64:128, :, csl])

        # ---- phase 2: attention ----
        eo = _register_exp_ops()
        from contextlib import ExitStack as _ES
        with _ES() as p2:
            psV = p2.enter_context(
                tc.tile_pool(name="psV", bufs=2, space="PSUM"))
            psS_cm = tc.tile_pool(name="psS", bufs=2, space="PSUM")
            psS = psS_cm.__enter__()

            pending = [None]

            def flush_mul():
                pv_, rc_, i_, s_, q0_ = pending[0]
                if s_ == 0:
                    nc.vector.tensor_mul(out=att_sb[0:64, i_, q0_:q0_ + qhw],
                                         in0=pv_[0:64, :], in1=rc_)
                else:
                    tmp = oddp.tile([64, qhw], BF16, tag="odd")
                    nc.vector.tensor_mul(out=tmp, in0=pv_[0:64, :], in1=rc_)
                    nc.sync.dma_start(out=att_sb[64:128, i_, q0_:q0_ + qhw],
                                      in_=tmp)
                pending[0] = None

            def scores_mm_for(h, q0, sc, kt):
                i, s = h // 2, h % 2
                if kt % 2 == 0 or not USE_SWAP:
                    lh, rh = kt_sb, qt_sb
                    half = slice(64 * s, 64 * s + 64)
                else:
                    lh, rh = ktw_sb, qtw_sb
                    half = slice(64 * (1 - s), 64 * (1 - s) + 64)
                for qq in range(qhw // 512):
                    nc.tensor.matmul(
                        sc[:, qq * 512:(qq + 1) * 512],
                        lh[half, i, kt * 128:(kt + 1) * 128],
                        rh[half, i, q0 + qq * 512:q0 + (qq + 1) * 512])

            def wo_job(t, pool, evict_all_vector=False):
                po = pool.tile([128, dim], F32, tag="pv" if pool is psV else "po")
                for c in range(NP):
                    lhsT = att_sb[:, c, t * 128:(t + 1) * 128]
                    # 512-col chunks: PSUM matmul outputs must not straddle
                    # a 2KB bank boundary
                    for o0 in (0, 512):
                        o1 = min(o0 + 512, dim)
                        nc.tensor.matmul(po[:, o0:o1], lhsT,
                                         wo_sb[:, c, o0:o1],
                                         start=(c == 0), stop=(c == NP - 1))
                ot = otp.tile([128, dim], out.dtype, tag="ot")
                # whole-tile eviction on ONE engine, alternating per tile:
                # the psO slot then frees after a single op (+1 sem), not
                # the max of two engines' halves
                if t % 2 == 0:
                    nc.vector.tensor_copy(out=ot, in_=po)
                else:
                    nc.scalar.copy(out=ot, in_=po)
                nc.sync.dma_start(out=out3[t], in_=ot)

            def unit(h, qh, dve_kts, dve_emit_j, dve_pv_j, handoff, nxt,
                     wo_tiles=()):
                i, s = h // 2, h % 2
                q0 = qh * qhw
                pv = psV.tile([65, qhw], F32, tag="pv")

                def pv_mm(p_t, kt, start, stop):
                    for qq in range(qhw // 512):
                        nc.tensor.matmul(
                            pv[:, qq * 512:(qq + 1) * 512],
                            v_sb[:, h, kt, :],
                            p_t[:, qq * 512:(qq + 1) * 512],
                            start=start, stop=stop)

                dve_pts = {}

                def emit_dve(kt):
                    sc = psV.tile([128, qhw], F32, tag="pv")
                    scores_mm_for(h, q0, sc, kt)
                    bt = bitp.tile([128, qhw], mybir.dt.int32, tag="bits")
                    nc.vector._custom_dve(eo["bits"], out=bt, in0=sc,
                                          s0=_RBIAS, s1=8388608.0,
                                          imm2=1065353216.0)
                    p_t = ppool.tile([128, qhw], BF16, tag="pd", bufs=2)
                    nc.vector._custom_dve(eo["frac"], out=p_t, in0=sc,
                                          in1=bt[:].bitcast(F32),
                                          s0=_RBIAS, s1=_PA, imm2=_PB)
                    dve_pts[kt] = p_t

                acts = [k for k in range(nt) if k not in dve_kts]
                prev = None
                for j, kt in enumerate(acts):
                    if j in dve_emit_j:
                        emit_dve(dve_kts[dve_emit_j[j]])
                    if j == 0 and handoff is not None:
                        sc = handoff
                    else:
                        sc = psS.tile([128, qhw], F32, tag="sc")
                        scores_mm_for(h, q0, sc, kt)
                    if j == 0 and pending[0] is not None:
                        flush_mul()
                    if j in dve_pv_j:
                        dkt = dve_kts[dve_pv_j[j]]
                        pv_mm(dve_pts.pop(dkt), dkt, start=False, stop=False)
                    if prev is not None:
                        pp, pkt = prev
                        pv_mm(pp, pkt, start=(j == 1), stop=False)
                    if wo_tiles and j in (4, 8):
                        widx = 0 if j == 4 else 1
                        if widx < len(wo_tiles):
                            wo_job(wo_tiles[widx], psV, evict_all_vector=True)
                    p_t = ppool.tile([128, qhw], BF16, tag="p")
                    nc.scalar.activation(out=p_t, in_=sc, func=AF.Exp,
                                         scale=LN2)
                    prev = (p_t, kt)
                assert not dve_pts
                # pre-emit the NEXT unit's first score matmul so the PE has
                # work while this unit's last exp drains
                handoff_out = None
                if nxt is not None:
                    nh_, nqh_, nkt0 = nxt
                    handoff_out = psS.tile([128, qhw], F32, tag="sc")
                    scores_mm_for(nh_, nqh_ * qhw, handoff_out, nkt0)
                pp, pkt = prev
                pv_mm(pp, pkt, start=False, stop=True)
                # denominator row 64 -> partition 0 via plain copy (custom
                # DVE ops can't partition-shift PSUM reads), reciprocal; the
                # multiply is deferred into the next unit so the gpsimd
                # broadcast latency hides behind that unit's first exps
                srow = rbpool.tile([1, qhw], F32, tag="srow")
                nc.vector.tensor_copy(out=srow, in_=pv[64:65, :])
                rrow = rbpool.tile([1, qhw], F32, tag="rrow")
                nc.vector.reciprocal_approx_fast(out=rrow, in_=srow)
                rc = rbpool.tile([64, qhw], F32, tag="rc", bufs=2)
                nc.gpsimd.partition_broadcast(rc, rrow)
                pending[0] = (pv, rc, i, s, q0)
                return handoff_out

            units = [(h, qh) for h in range(NH) for qh in range(nqh)]
            wo_sched = {}  # in-unit Wo jobs measured slower (psV contention)
            # the final unit runs fewer DVE key-tiles so the DVE queue is
            # drained by the time phase 3 needs the last normalize multiply
            cfg_main = (DVE_KTS, DVE_EMIT_J, DVE_PV_J)
            cfg_last = ((2, 5), {0: 0, 3: 1}, {3: 0, 6: 1})
            handoff = None
            for u, (h, qh) in enumerate(units):
                dkts, dej, dpj = cfg_last if u == len(units) - 1 else cfg_main
                nxt = None
                if u + 1 < len(units):
                    nh_, nqh_ = units[u + 1]
                    ndkts = cfg_last[0] if u + 1 == len(units) - 1 else DVE_KTS
                    nacts = [k for k in range(nt) if k not in ndkts]
                    nxt = (nh_, nqh_, nacts[0])
                handoff = unit(h, qh, dkts, dej, dpj, handoff, nxt,
                               wo_tiles=wo_sched.get(u, ()))
            # close the score pool BEFORE the final deferred multiply: its
            # close-barrier only waits on the last exp, so its 4 banks are
            # free for the output projection while psV (the pv accumulator)
            # is still pinned by the final normalize - Wo tiles 0..14 don't
            # need the last head's data and start immediately
            psS_cm.__exit__(None, None, None)
            flush_mul()

            # ---- phase 3: output projection (2-deep in the freed banks) ----
            with tc.tile_pool(name="psO", bufs=2, space="PSUM") as psO:
                for t in range(nt):
                    wo_job(t, psO)
                if dbg:
                    nc.sync.dma_start(out=dbg["xnT"], in_=xnT)
                    nc.sync.dma_start(out=dbg["kt"], in_=kt_sb)
                    nc.sync.dma_start(out=dbg["qt"], in_=qt_sb)
                    nc.sync.dma_start(out=dbg["v"], in_=v_sb)
                    nc.sync.dma_start(out=dbg["att"], in_=att_sb)


_NC_CACHE = {}


def _get_nc():
    if "nc" not in _NC_CACHE:
        _NC_CACHE["nc"] = build_graph()
    return _NC_CACHE["nc"]


def make_in_maps(x, gamma, Wq, Wk, Wv, Wo):
    """Host-side sharding: core c -> batch c//2, head-group c%2."""
    import ml_dtypes
    bf16 = ml_dtypes.bfloat16
    g = (np.asarray(gamma, np.float32) + 1.0)
    scale = DH ** -0.5 * LOG2E  # scores computed in log2 domain
    Wq_eff = np.asarray(Wq, np.float32) * g[None, :] * scale
    Wk_eff = np.asarray(Wk, np.float32) * g[None, :]
    Wv_eff = np.asarray(Wv, np.float32)
    Wo_eff = np.asarray(Wo, np.float32)
    hg_maps = []
    for hg in range(2):
        r0, r1 = hg * HCOLS, (hg + 1) * HCOLS
        hg_maps.append({
            "wqt": np.ascontiguousarray(Wq_eff[r0:r1, :].T).astype(bf16),
            "wkt": np.ascontiguousarray(Wk_eff[r0:r1, :].T).astype(bf16),
            "wvt": np.ascontiguousarray(Wv_eff[r0:r1, :].T).astype(bf16),
            "wot": np.ascontiguousarray(Wo_eff[:, r0:r1].T).astype(bf16),
        })
    in_maps = []
    for c in range(NCORES):
        b, hg = c // 2, c % 2
        m = dict(hg_maps[hg])
        m["x"] = np.ascontiguousarray(np.asarray(x, np.float32)[b])
        in_maps.append(m)
    return in_maps


def _run(inputs, trace=False, trace_kwargs=None):
    nc = _get_nc()
    in_maps = make_in_maps(**inputs)
    res = run_bass_kernel_spmd(nc, in_maps, core_ids=list(range(NCORES)),
                               trace=trace, **(trace_kwargs or {}))
    out = np.empty((B, N, DIM), np.float32)
    for b in range(B):
        out[b] = (res.results[2 * b]["out"].astype(np.float32)
                  + res.results[2 * b + 1]["out"].astype(np.float32))
    return out, res


def kernel(x, gamma, Wq, Wk, Wv, Wo):
    out, _ = _run(dict(x=x, gamma=gamma, Wq=Wq, Wk=Wk, Wv=Wv, Wo=Wo))
    return out
